# revision 33
# baseline (speedup 1.0000x reference)
"""Fused multi-head attention with dropout for Trainium2 (Bass/Tile), 8-core SPMD.

Problem: out = dropout(softmax(Q @ K^T * scale)) @ V
  Q/K/V: [64, 2048, 64] fp32, dropout_mask: [64, 2048, 2048] fp32, p = 0.5.

Sharding: the 64 batch*heads are split across 8 NeuronCores (8 heads/core),
no cross-device communication.

Per-head device algorithm (head-local, S = 2048, D = 64):
  Scores are computed TRANSPOSED, S^T[k, q] = K @ Q^T, so that softmax rows
  (over k) land on the PSUM partition axis and the PV product needs no
  on-chip transpose: O^T[d, q] = sum_k V[k, d] * P[k, q] accumulates in PSUM.
  exp is taken without max-subtraction (|scores| <= ~50, fp32 exp safe).

  Engine balance (per core, 33.5M score elems):
  - PE: QK fp32r (109us) + PV bf16 (109us) + denominator ones-matmuls.
  - ACT: exp fp32(PSUM) -> bf16(SBUF), ~274us - co-bottleneck.
  - DVE: mask-mult in bf16 (2x mode, 177us) + pair-sum adds + tails.
  The probability pipeline is bf16 end-to-end (exp output, mask, PV moving,
  denominator moving): halves DVE time vs fp32 and leaves PE cost unchanged.
  The softmax denominator sum_k exp is a ones-stationary matmul; for the
  first DEN_PAIRS chunk-pairs per block the two exp tiles are pre-summed on
  DVE (bf16 2x add) so the ones-matmul runs on the pair-sum at half the
  moving-column cost - DEN_PAIRS trades PE cycles against DVE cycles.

Host-side (sharding prep): Q, K fed transposed ([D, S] per head, fp32),
V packed to [128, (S/128)*D] bf16, the keep-mask (mask >= p) scaled by
1/(1-p) shipped as bf16 {0, 2} transposed to [k, q]; output [D, S] fp32
transposed back on gather.
"""

import numpy as np
from contextlib import ExitStack

import concourse.bass as bass
import concourse.bacc as bacc
import concourse.tile as tile
import concourse.mybir as mybir
from concourse.bass_utils import run_bass_kernel_spmd

N_CORES = 8
B, S, D = 64, 2048, 64
HPC = B // N_CORES  # heads per core
KP = 128            # k-chunk size (PSUM partition dim)
NQ = 512            # matmul moving free-dim tile (one fp32 PSUM bank)
DROP_P = 0.5
DEN_PAIRS = 8       # chunk-pairs per block whose denominator is DVE pre-summed (0..8)


def build_program(n_heads=HPC, seq=S, d=D, scale=1.0, den_pairs=DEN_PAIRS, reps=1,
                  mask_gpsimd=True, shared_bank=True, early_stage=True):
    f32 = mybir.dt.float32
    bf16 = mybir.dt.bfloat16
    # float32r: same fp32 bytes, PE streams 1 col/cycle (vs 4 for fp32) at
    # ~tf32 precision. Used for QK only; the P/V pipeline runs bf16.
    fmm = mybir.dt.float32r
    n_kc = seq // KP
    QL = min(1024, seq)  # q-slice width processed per PSUM accumulator
    n_qh = seq // QL
    n_j = QL // NQ

    nc = bacc.Bacc("TRN2", target_bir_lowering=False, debug=False)
    qt_d = nc.dram_tensor("qt", [n_heads, d, seq], fmm, kind="ExternalInput").ap()
    kt_d = nc.dram_tensor("kt", [n_heads, d, seq], fmm, kind="ExternalInput").ap()
    # V packed [V_0 | 64 zero cols | V_1 .. V_15]: chunk 0's stationary is the
    # 128-wide [V_0 | 0] slab, whose matmul is the single start=True of the
    # whole PSUM accumulator bank (hardware clears has_written bank-wide on
    # start, so O^T rows and denominator rows sharing a bank must be started
    # by ONE matmul; the zero half initializes the denominator rows). A
    # 128-col stationary costs the same as 64 - matmul time is moving-only.
    vp_d = nc.dram_tensor(
        "vp", [n_heads, KP, (n_kc + 2) * d], bf16, kind="ExternalInput"
    ).ap()
    mt_d = nc.dram_tensor("mt", [n_heads, seq, seq], bf16, kind="ExternalInput").ap()
    ot_d = nc.dram_tensor("ot", [n_heads, d, seq], f32, kind="ExternalOutput").ap()

    # Software-pipelined emission over a flat list of (head, q-slice) blocks:
    # per chunk c the program order is [dma mask(next)] [exp(c)] [QK(next)]
    # [mask-mult(c)] [pair-add] [PV/denom(c)], so each engine's in-order
    # stream never waits on the current chunk's cross-engine chain.
    blocks = [(h, qh) for h in range(n_heads) for qh in range(n_qh)] * reps

    with tile.TileContext(nc) as tc:
        with ExitStack() as ctx:
            const = ctx.enter_context(tc.tile_pool(name="const", bufs=1))
            qkv = ctx.enter_context(tc.tile_pool(name="qkv", bufs=2))
            mpool = ctx.enter_context(tc.tile_pool(name="mask", bufs=10))
            p0pool = ctx.enter_context(tc.tile_pool(name="p0", bufs=4))
            pdpool = ctx.enter_context(tc.tile_pool(name="pd", bufs=3))
            pspool = ctx.enter_context(tc.tile_pool(name="ps", bufs=2))
            opool = ctx.enter_context(tc.tile_pool(name="o", bufs=2))
            # PSUM budget (8 banks, 16KB/partition): st 2x[128,1024] +
            # oaccd 2x[128,1024]. oaccd rows 0..63 hold O^T, rows 64..127 the
            # replicated softmax denominator - sharing one tile (instead of a
            # separate [64,QL] pden) is what frees the banks to double-buffer
            # the accumulator, which removes the block-boundary PE stall.
            pst = ctx.enter_context(
                tc.tile_pool(name="pst", bufs=2, space=bass.MemorySpace.PSUM)
            )
            pacc = ctx.enter_context(
                tc.tile_pool(
                    name="pacc", bufs=2 if shared_bank else 1,
                    space=bass.MemorySpace.PSUM,
                )
            )
            pden = (
                None
                if shared_bank
                else ctx.enter_context(
                    tc.tile_pool(name="pden", bufs=1, space=bass.MemorySpace.PSUM)
                )
            )

            # d identical ones columns: the denominator matmul emits sum_k p
            # already replicated across the d output partitions (needed on
            # partitions 0..d-1 for the recip/normalize tail).
            ones_bf = const.tile([KP, d], bf16)
            nc.vector.memset(ones_bf[:], 1.0)

            head_tiles: dict = {}

            def load_head(h):
                kt_sb = qkv.tile([d, seq], fmm, tag="kt")
                nc.sync.dma_start(kt_sb[:], kt_d[h])
                qt_sb = qkv.tile([d, seq], fmm, tag="qt")
                nc.sync.dma_start(qt_sb[:], qt_d[h])
                v_sb = qkv.tile([KP, (n_kc + 2) * d], bf16, tag="v")
                nc.sync.dma_start(v_sb[:], vp_d[h])
                head_tiles[h] = (qt_sb, kt_sb, v_sb)

            mk_tiles: dict = {}
            st_tiles: dict = {}

            def dma_mk(b, c):
                # issued from the (otherwise idle) Pool/GPSIMD sequencer:
                # 25ns dispatch vs 565ns on SP, which otherwise becomes the
                # bottleneck engine from DMA dispatch alone.
                h, qh = blocks[b]
                q0 = qh * QL
                t = mpool.tile([KP, QL], bf16, tag="mk")
                eng = nc.gpsimd if mask_gpsimd else nc.sync
                eng.dma_start(t[:], mt_d[h, c * KP : (c + 1) * KP, q0 : q0 + QL])
                mk_tiles[(b, c)] = t

            def qk(b, c):
                h, qh = blocks[b]
                q0 = qh * QL
                qt_sb, kt_sb, _ = head_tiles[h]
                t = pst.tile([KP, QL], f32, tag="st")
                for j in range(n_j):
                    nc.tensor.matmul(
                        t[:, j * NQ : (j + 1) * NQ],
                        kt_sb[:, c * KP : (c + 1) * KP],
                        qt_sb[:, q0 + j * NQ : q0 + (j + 1) * NQ],
                        start=True,
                        stop=True,
                    )
                st_tiles[(b, c)] = t

            # Startup staging: small early tiles for head 0's first chunks go
            # out on the Activation DGE queue while the bulk head-0 tiles
            # (v first - needed by the first PV) stream on SP, so the first
            # QK issues ~2us in instead of ~8us.
            if early_stage:
                kt_e = const.tile([d, 4 * KP], fmm)
                nc.sync.dma_start(kt_e[:], kt_d[0][:, 0 : 4 * KP])
                qt_e = const.tile([d, QL], fmm)
                nc.scalar.dma_start(qt_e[:], qt_d[0][:, 0:QL])

            def load_head0():
                v_sb = qkv.tile([KP, (n_kc + 2) * d], bf16, tag="v")
                nc.sync.dma_start(v_sb[:], vp_d[0])
                kt_sb = qkv.tile([d, seq], fmm, tag="kt")
                nc.sync.dma_start(kt_sb[:], kt_d[0])
                qt_sb = qkv.tile([d, seq], fmm, tag="qt")
                nc.sync.dma_start(qt_sb[:], qt_d[0])
                head_tiles[0] = (qt_sb, kt_sb, v_sb)

            def qk_early(b, c):
                if not early_stage:
                    return qk(b, c)
                t = pst.tile([KP, QL], f32, tag="st")
                for j in range(n_j):
                    nc.tensor.matmul(
                        t[:, j * NQ : (j + 1) * NQ],
                        kt_e[:, c * KP : (c + 1) * KP],
                        qt_e[:, j * NQ : (j + 1) * NQ],
                        start=True,
                        stop=True,
                    )
                st_tiles[(b, c)] = t

            load_head0()
            dma_mk(0, 0)
            qk_early(0, 0)

            # Deferred tail of the previous block, in two stages emitted a
            # few chunks into the NEXT block so the DVE in-order stream never
            # makes the new block's first PV wait:
            #   A: recip of the denominator in place on partitions 64..127,
            #      then an SBUF->SBUF DMA hops it to partitions 0..63 (DVE
            #      lanes cannot cross partitions; DMA can).
            #   B: normalize O^T rows 0..63 by the hopped reciprocal, ship.
            pending_a = None
            pending_b = None

            def emit_tail_a(tail):
                if not shared_bank:
                    oaccd_p, oden_p, h_p, qh_p = tail
                    rb = opool.tile([d, QL], f32, tag="rblo")
                    nc.vector.reciprocal_approx_fast(rb[:], oden_p[:])
                    return (oaccd_p, rb, h_p, qh_p)
                oaccd_p, h_p, qh_p = tail
                # DVE cannot read PSUM at a partition offset (HW bug: offset-64
                # reads return garbage) - so the recip runs over the FULL 128
                # partitions at offset 0 (rows 0:63 produce unused junk, same
                # cost: partitions are parallel) and the DMA hop picks rows
                # 64:128, which DMA reads fine.
                rb_hi = opool.tile([2 * d, QL], f32, tag="rbhi")
                nc.vector.reciprocal_approx_fast(rb_hi[:], oaccd_p[:])
                rb_lo = opool.tile([d, QL], f32, tag="rblo")
                nc.sync.dma_start(rb_lo[:], rb_hi[d : 2 * d, :])
                return (oaccd_p, rb_lo, h_p, qh_p)

            def emit_tail_b(tail):
                oaccd_p, rb_lo, h_p, qh_p = tail
                out_sb = opool.tile([d, QL], f32, tag="out")
                nc.vector.tensor_mul(out_sb[:], oaccd_p[0:d, :], rb_lo[:])
                nc.sync.dma_start(
                    ot_d[h_p, :, qh_p * QL : qh_p * QL + QL], out_sb[:]
                )

            # PV/denominator of chunk i are EMITTED one chunk behind (at slot
            # i+1): by then exp(i) and mult(i) have long finished, so the PE
            # stream never blocks on the QK->exp->mult chain of the current
            # chunk.
            pending_back = None

            def make_back(oaccd, v_sb, pd, den_mov, first, last, c, oden=None):
                def back():
                    if not shared_bank:
                        for j in range(n_j):
                            nc.tensor.matmul(
                                oaccd[0:d, j * NQ : (j + 1) * NQ],
                                v_sb[:, (0 if first else (c + 1) * d) :][:, 0:d],
                                pd[:, j * NQ : (j + 1) * NQ],
                                start=first,
                                stop=last,
                            )
                        if den_mov is not None:
                            for j in range(n_j):
                                nc.tensor.matmul(
                                    oden[0][:, j * NQ : (j + 1) * NQ],
                                    ones_bf[:],
                                    den_mov[:, j * NQ : (j + 1) * NQ],
                                    start=not oden[1],
                                    stop=last,
                                )
                            oden[1] = True
                        return
                    # Bank accumulation-group protocol: exactly ONE start
                    # (chunk 0) and ONE stop (chunk 15) per bank, both via
                    # 128-wide [V_c | 0] stationaries covering all partitions
                    # (hardware clears has_written bank-wide on start, so the
                    # denominator matmuls all ride start=False; the zero half
                    # initializes / no-op-closes the denominator rows at zero
                    # cost - matmul time is moving-columns only). On the last
                    # chunk the denominator is emitted first so the stop is
                    # the bank's final matmul.
                    def emit_den():
                        for j in range(n_j):
                            nc.tensor.matmul(
                                oaccd[d : 2 * d, j * NQ : (j + 1) * NQ],
                                ones_bf[:],
                                den_mov[:, j * NQ : (j + 1) * NQ],
                                start=False,
                                stop=False,
                            )

                    if last and den_mov is not None:
                        emit_den()
                    for j in range(n_j):
                        if first or last:
                            nc.tensor.matmul(
                                oaccd[:, j * NQ : (j + 1) * NQ],
                                v_sb[:, (0 if first else (c + 1) * d) :][:, 0 : 2 * d],
                                pd[:, j * NQ : (j + 1) * NQ],
                                start=first,
                                stop=last,
                            )
                        else:
                            nc.tensor.matmul(
                                oaccd[0:d, j * NQ : (j + 1) * NQ],
                                v_sb[:, (c + 1) * d : (c + 2) * d],
                                pd[:, j * NQ : (j + 1) * NQ],
                                start=False,
                                stop=False,
                            )
                    if not last and den_mov is not None:
                        emit_den()

                return back

            for b, (h, qh) in enumerate(blocks):
                _, _, v_sb = head_tiles[h]
                if shared_bank:
                    oaccd = pacc.tile([2 * d, QL], f32, tag="oaccd")
                    oden_state = None
                else:
                    oaccd = pacc.tile([d, QL], f32, tag="oaccd")
                    oden_t = pden.tile([d, QL], f32, tag="oden")
                    oden_state = [oden_t, False]
                p0_prev = None
                for c in range(n_kc):
                    nxt = (b, c + 1) if c + 1 < n_kc else (b + 1, 0)
                    if nxt[0] >= len(blocks):
                        nxt = None
                    # prefetch the next head's tensors halfway through its
                    # predecessor's last block
                    if (
                        c == 2
                        and b + 1 < len(blocks)
                        and blocks[b + 1][0] != h
                    ):
                        load_head(blocks[b + 1][0])
                    if nxt is not None:
                        dma_mk(*nxt)

                    st = st_tiles.pop((b, c))
                    p0 = p0pool.tile([KP, QL], bf16, tag="p0")
                    nc.scalar.activation(
                        p0[:], st[:], mybir.ActivationFunctionType.Exp, scale=scale
                    )
                    if nxt is not None:
                        if nxt == (0, 1):
                            qk_early(*nxt)
                        else:
                            qk(*nxt)
                    mk = mk_tiles.pop((b, c))
                    pd = pdpool.tile([KP, QL], bf16, tag="pd")
                    nc.vector.tensor_tensor(pd[:], mk[:], p0[:], mybir.AluOpType.mult)
                    if c == 1 and pending_a is not None:
                        pending_b = emit_tail_a(pending_a)
                        pending_a = None
                    elif c == 3 and pending_b is not None:
                        emit_tail_b(pending_b)
                        pending_b = None

                    first, last = c == 0, c == n_kc - 1
                    paired = (c // 2) < den_pairs
                    den_mov = None
                    if paired:
                        if c % 2 == 0:
                            p0_prev = p0
                        else:
                            ps = pspool.tile([KP, QL], bf16, tag="ps")
                            nc.vector.tensor_tensor(
                                ps[:], p0_prev[:], p0[:], mybir.AluOpType.add
                            )
                            p0_prev = None
                            den_mov = ps
                    else:
                        den_mov = p0

                    if pending_back is not None:
                        pending_back()
                    pending_back = make_back(oaccd, v_sb, pd, den_mov, first, last, c, oden_state)

                # out = oacc * (1 / sum_k exp); the 1/(1-p) dropout rescale
                # rides in the bf16 mask values {0, 2}.
                if shared_bank:
                    pending_a = (oaccd, h, qh)
                else:
                    pending_a = (oaccd, oden_state[0], h, qh)

            pending_back()
            pending_back = None
            if not shared_bank:
                pending_b = emit_tail_a(pending_a)
                emit_tail_b(pending_b)
            else:
                # Final block: j-granular tail so recip/hop/normalize/store
                # pipeline instead of serializing ~10us of end drain.
                oaccd_p, h_p, qh_p = pending_a
                rb_hi = opool.tile([2 * d, QL], f32, tag="rbhi")
                rb_lo = opool.tile([d, QL], f32, tag="rblo")
                out_sb = opool.tile([d, QL], f32, tag="out")
                for j in range(n_j):
                    jj = slice(j * NQ, (j + 1) * NQ)
                    nc.vector.reciprocal_approx_fast(rb_hi[:, jj], oaccd_p[:, jj])
                    (nc.sync if j % 2 == 0 else nc.scalar).dma_start(
                        rb_lo[:, jj], rb_hi[d : 2 * d, jj]
                    )
                for j in range(n_j):
                    jj = slice(j * NQ, (j + 1) * NQ)
                    nc.vector.tensor_mul(out_sb[:, jj], oaccd_p[0:d, jj], rb_lo[:, jj])
                    (nc.sync if j % 2 == 0 else nc.scalar).dma_start(
                        ot_d[h_p, :, qh_p * QL + j * NQ : qh_p * QL + (j + 1) * NQ],
                        out_sb[:, jj],
                    )
            pending_a = pending_b = None

    nc.compile()
    return nc


_CACHE: dict = {}


def _get_program(scale: float, reps: int = 1, **kw):
    key = (float(scale), reps, tuple(sorted(kw.items())))
    if key not in _CACHE:
        _CACHE[key] = build_program(scale=key[0], reps=reps, **kw)
    return _CACHE[key]


def make_in_maps(query, key, value, dropout_mask):
    """Shard + relayout the full inputs into the 8 per-core input maps."""
    import ml_dtypes

    query = np.asarray(query, dtype=np.float32)
    key = np.asarray(key, dtype=np.float32)
    value = np.asarray(value, dtype=np.float32)
    dropout_mask = np.asarray(dropout_mask, dtype=np.float32)
    in_maps = []
    for c in range(N_CORES):
        sl = slice(c * HPC, (c + 1) * HPC)
        qt = np.ascontiguousarray(query[sl].transpose(0, 2, 1))
        kt = np.ascontiguousarray(key[sl].transpose(0, 2, 1))
        vk = value[sl].reshape(HPC, S // KP, KP, D).transpose(0, 2, 1, 3)
        vp = np.zeros((HPC, KP, S // KP + 2, D), dtype=np.float32)
        vp[:, :, 0] = vk[:, :, 0]   # V_0; [:, :, 1] stays zero (bank-start pad)
        vp[:, :, 2:-1] = vk[:, :, 1:]  # V_1..V_15; trailing pad for the stop
        vp = vp.reshape(HPC, KP, (S // KP + 2) * D).astype(ml_dtypes.bfloat16)
        mt = np.ascontiguousarray(dropout_mask[sl].transpose(0, 2, 1))
        mt = ((mt >= DROP_P) * np.float32(1.0 / (1.0 - DROP_P))).astype(
            ml_dtypes.bfloat16
        )
        in_maps.append({"qt": qt, "kt": kt, "vp": vp, "mt": mt})
    return in_maps


def run(query, key, value, scale_factor, dropout_mask, trace=False, **trace_kwargs):
    scale = float(np.asarray(scale_factor).reshape(()))
    nc = _get_program(scale)
    in_maps = make_in_maps(query, key, value, dropout_mask)
    res = run_bass_kernel_spmd(
        nc, in_maps, core_ids=list(range(N_CORES)), trace=trace, **trace_kwargs
    )
    outs = [res.results[c]["ot"].transpose(0, 2, 1) for c in range(N_CORES)]
    full = np.ascontiguousarray(np.concatenate(outs, axis=0), dtype=np.float32)
    return full, res


def kernel(query, key, value, scale_factor, dropout_mask):
    out, _ = run(query, key, value, scale_factor, dropout_mask, trace=False)
    return out
